# revision 8
# baseline (speedup 1.0000x reference)
"""MetaBaseline (retrieval_knn) Trainium2 kernel — v3.

Problem: E=256 episodes; per episode:
  shot_sum[W,D], shot_mean = mean over S shots
  dist[W,Q]   = ||shot_mean_w - q_q||_2
  weights     = softmax(-dist, axis=Q)
  pooled[W,D] = weights @ x_query
  proto       = l2norm(shot_sum + 37*pooled)       (the /42 cancels in l2norm)
  logits[Q,W] = temp * l2norm(x_query) @ proto.T

Sharding: pure data parallel over E across 8 NeuronCores (32 episodes/core).
On-device layout: blocks of 4 episodes packed on the partition dim at
32-partition offsets (col-tiled matmuls), so softmax/activation work runs on
[128, Q] tiles serving 4 episodes at once.

v3 changes vs v2 (54us):
  - ALL query streams in fp8 e3m4 (4-bit mantissa): q scaled by 2 on host
    (fits e3m4 range +-15.5), 1/2 folded into mTs (-mean), ALPHA (18.5) and
    the host-side output scale. Input DMA drops 18.4MB -> ~12.4MB/core.
    rel_fro ~1.3e-2 (numpy study) < 2e-2.
  - matmuls issued col-group-interleaved (c-outer, j-inner) so the 4
    episodes' chains overlap in the PE array (32-col tile concurrency).
    Since a start=True clears has_written for the WHOLE psum bank, each
    phase opens with a single full-width (M=128) start matmul that does
    useful work: G starts with a K=8 block-ones matmul folding nq2 hi/lo;
    the proto accumulation starts with a K=80 block-identity matmul folding
    ssum; the logits matmul starts with a K=1 zero matmul. All chain
    matmuls then run start=False in any order (overwrite-where-unset).
  - PE transposes replaced by HWDGE xbar DMA transposes (SBUF->SBUF,
    [128,128] bf16): w37 (3) and prs (4). The l2norm rstd is folded into
    the final per-partition logits scale, so the transpose runs on the
    UNnormalized prs and doesn't wait for the norm chain.

Device pipeline per block b (episodes j=0..3 at partitions 32j..32j+20):
  1. G psum[128,Q]    = ones8.T @ nqhl (K=8, full-width start)
                        + mTs.T @ qT   (4 K-chunks, col-tiled, fp8 x fp8)
  2. ACT: Ln -> Exp(0.5) = dist -> Exp(-dist) with accum_out sums
  3. w37 = exp(-dist) * (18.5/sums)       (DVE tensor_scalar, out bf16)
  4. wT  = 3x DMA xbar transpose [128,128] of w37
  5. pr psum[128,512] = identE.T @ ssum (K=80, full-width start)
                        + wT.T @ q8n (col-tiled)
  6. prsb = bf16 copy; ACT Square+accum -> n2; rstd = exp(-0.5*ln(n2))
  7. prT = 4x DMA xbar transpose [128,128] of prsb
  8. lgT psum[128,Q] = zcol.T @ x (K=1 zero start) + prT.T @ qT (4 K-chunks)
     lgsb = lgT * rstd (DVE, folds proto l2norm); DMA out as [128, b, Q].
     Host does final transpose + temp/||q|| scale.
"""
import sys

sys.path.insert(0, "/opt/trn_rl_repo")

import numpy as np
import ml_dtypes

import concourse.bass as bass
import concourse.tile as tile
from concourse import bacc, mybir
from concourse.bass_utils import run_bass_kernel_spmd

bf16 = mybir.dt.bfloat16
f32 = mybir.dt.float32
f8e3 = mybir.dt.float8e3

E, W, S, Q, D = 256, 20, 5, 300, 512
ALPHA = 37.0
QS = 2.0              # host-side scale on x_query before e3m4 cast
NCORES = 8
EL = E // NCORES      # 32 episodes per core
BLK = 4               # episodes per block (packed at 32-partition offsets)
NBLK = EL // BLK      # 8 blocks
DC = D // 128         # 4 K-chunks over D
QCH = [(0, 128), (128, 128), (256, Q - 256)]  # q chunks (offset, count)

_BUILT = {}
import os as _os
_SEQ_DEFAULT = bool(int(_os.environ.get("KSEQ", "0")))


def _pin_act_table_set():
    """Make Bacc's ACT-table-load pass pick one covering set for Ln/Exp/Square.

    The pass walks activations and loads the first set containing the needed
    function; Ln's first set lacks Exp and vice versa, so alternating
    Ln/Exp/Square thrashes ACT_TABLE_LOAD (~1.3us each). Hide those functions
    from every set except natural_log_exp_and_others (set *indices* are
    preserved — contents of the real act_info.json are untouched).
    """
    import concourse.bacc as bacc_mod
    from concourse import hw_specs

    if getattr(bacc_mod, "_act_tables_pinned", False):
        return
    orig = hw_specs.get_activation_tables
    pin = {
        mybir.ActivationFunctionType.Ln,
        mybir.ActivationFunctionType.Exp,
        mybir.ActivationFunctionType.Square,
    }
    keep = "natural_log_exp_and_others"

    def pinned(arch):
        tabs = orig(arch)
        return {
            name: set(fns) if name == keep else (set(fns) - pin)
            for name, fns in tabs.items()
        }

    bacc_mod.get_activation_tables = pinned
    bacc_mod._act_tables_pinned = True


def _build(reps=1):
    _pin_act_table_set()
    nc = bacc.Bacc("TRN2", target_bir_lowering=False, debug=False)

    ssum = nc.declare_dram_parameter("ssum", [BLK * W, NBLK, D], bf16, isOutput=False)
    qnat = nc.declare_dram_parameter("qnat", [Q, EL, D], f8e3, isOutput=False)
    qT = nc.declare_dram_parameter("qT", [128, NBLK, DC * BLK * Q], f8e3, isOutput=False)
    mTs = nc.declare_dram_parameter("mTs", [128, NBLK, DC * BLK * W], f8e3, isOutput=False)
    nq2hl = nc.declare_dram_parameter("nq2hl", [2 * BLK, NBLK, Q], bf16, isOutput=False)
    ones8d = nc.declare_dram_parameter("ones8d", [2 * BLK, 128], bf16, isOutput=False)
    identEd = nc.declare_dram_parameter("identEd", [BLK * W, 128], bf16, isOutput=False)
    zcold = nc.declare_dram_parameter("zcold", [1, 128], bf16, isOutput=False)
    nm2b = nc.declare_dram_parameter("nm2b", [128, NBLK], f32, isOutput=False)
    outT = nc.declare_dram_parameter("outT", [128, NBLK, Q], bf16, isOutput=True)

    with tile.TileContext(nc) as tc:
        with tc.tile_pool(name="const", bufs=1) as const, \
             tc.tile_pool(name="inp", bufs=3) as inp, \
             tc.tile_pool(name="mid", bufs=3) as mid, \
             tc.tile_pool(name="psA", bufs=3, space="PSUM") as psA, \
             tc.tile_pool(name="psB", bufs=2, space="PSUM") as psB:

            # ---- constants (loaded once) ----
            nm2b_t = const.tile([128, NBLK], f32)
            nc.sync.dma_start(out=nm2b_t, in_=nm2b[:, :])
            ones8_t = const.tile([2 * BLK, 128], bf16)
            nc.sync.dma_start(out=ones8_t, in_=ones8d[:, :])
            identE_t = const.tile([BLK * W, 128], bf16)
            nc.sync.dma_start(out=identE_t, in_=identEd[:, :])
            zcol_t = const.tile([1, 128], bf16)
            nc.sync.dma_start(out=zcol_t, in_=zcold[:, :])
            identE_t = const.tile([BLK * W, 128], bf16)
            nc.sync.dma_start(out=identE_t, in_=identEd[:, :])
            # PE warmup: dense dummy matmuls during the DMA ramp so the HAM
            # un-throttles (1.2 -> 2.4 GHz) before the first real block.
            wups = psA.tile([128, 128], f32, tag="g")
            for _ in range(40):
                nc.tensor.matmul(wups, identE_t[:, 0:128], identE_t[:, 0:128],
                                 start=True, stop=True)
            wupd = const.tile([128, 128], bf16)
            nc.vector.tensor_copy(wupd, wups)

            def s1(b, st):
                """inputs; G psum (nq2 start-fold + dist matmuls); w37."""
                e0 = b * BLK
                mTs_t = inp.tile([128, DC, BLK, W], f8e3, tag="mTs", bufs=8)
                nc.sync.dma_start(
                    out=mTs_t,
                    in_=mTs[:, b, :].rearrange("p (c j w) -> p c j w", c=DC, j=BLK),
                )
                qTt = inp.tile([128, DC, BLK, Q], f8e3, tag="qTt", bufs=8)
                nc.sync.dma_start(
                    out=qTt,
                    in_=qT[:, b, :].rearrange("p (c j q) -> p c j q", c=DC, j=BLK),
                )
                ssum_t = inp.tile([BLK * W, D], bf16, tag="ssum", bufs=4)
                nc.gpsimd.dma_start(out=ssum_t, in_=ssum[:, b, :])
                qn_t = []
                for ci, (q0, cnt) in enumerate(QCH):
                    t = inp.tile([128, BLK, D], f8e3, tag=f"qn{ci}", bufs=4)
                    eng = nc.scalar
                    eng.dma_start(out=t[0:cnt], in_=qnat[q0:q0 + cnt, e0:e0 + BLK, :])
                    qn_t.append(t)
                nqh_t = inp.tile([2 * BLK, Q], bf16, tag="nqh", bufs=6)
                nc.sync.dma_start(out=nqh_t, in_=nq2hl[:, b, :])
                if seq:
                    nqh2_t = inp.tile([2, BLK, Q], bf16, tag="nqh2", bufs=6)
                    nc.sync.dma_start(
                        out=nqh2_t,
                        in_=nq2hl[:, b, :].rearrange("(j p) q -> p j q", p=2))
                    ssum2_t = inp.tile([W, BLK, D], bf16, tag="ssum2", bufs=4)
                    nc.gpsimd.dma_start(
                        out=ssum2_t,
                        in_=ssum[:, b, :].rearrange("(j p) d -> p j d", p=W))
                    st.update(nqh2_t=nqh2_t, ssum2_t=ssum2_t)
                st.update(qTt=qTt, qn_t=qn_t, ssum_t=ssum_t, nqh_t=nqh_t)

                g = psA.tile([128, Q], f32, tag="g")
                # full-width start: clears the bank, writes nq2 fold rows
                nc.tensor.matmul(g, ones8_t, nqh_t, start=True, stop=False, skip_group_check=True)
                for c in range(DC):
                    for j in range(BLK):
                        nc.tensor.matmul(
                            g[32 * j:32 * j + W, :],
                            mTs_t[:, c, j, :], qTt[:, c, j, :],
                            start=False, stop=(c == DC - 1 and j == BLK - 1),
                            tile_position=(0, 32 * j),
                            skip_group_check=True,
                        )
                lnv = mid.tile([128, Q], f32, tag="lnv", bufs=4)
                nc.scalar.activation(out=lnv, in_=g,
                                     func=mybir.ActivationFunctionType.Ln,
                                     bias=nm2b_t[:, b:b + 1], scale=1.0)
                dist = mid.tile([128, Q], f32, tag="dist", bufs=4)
                nc.scalar.activation(out=dist, in_=lnv,
                                     func=mybir.ActivationFunctionType.Exp,
                                     bias=0.0, scale=0.5)
                wexp = mid.tile([128, Q], f32, tag="wexp", bufs=4)
                sums = mid.tile([128, 1], f32, tag="sums")
                nc.scalar.activation(out=wexp, in_=dist,
                                     func=mybir.ActivationFunctionType.Exp,
                                     bias=0.0, scale=-1.0, accum_out=sums)
                recip = mid.tile([128, 1], f32, tag="recip")
                nc.vector.reciprocal(recip, sums)
                w37 = mid.tile([128, 384], bf16, tag="w37")
                nc.vector.tensor_scalar(
                    out=w37[:, 0:Q], in0=wexp, scalar1=recip, scalar2=ALPHA / QS,
                    op0=mybir.AluOpType.mult, op1=mybir.AluOpType.mult,
                )
                nc.vector.memset(w37[:, Q:384], 0.0)
                st["w37"] = w37

            def s2a(b, st):
                """wT via DMA xbar transposes (end-of-iteration)."""
                w37 = st["w37"]
                wTsb = []
                for ci in range(3):
                    t = mid.tile([128, 128], bf16, tag=f"wTsb{ci}")
                    nc.scalar.dma_start(
                        out=t, in_=w37[:, 128 * ci:128 * (ci + 1)], transpose=True)
                    wTsb.append(t)
                st["wTsb"] = wTsb

            def s2b(b, st):
                """proto accumulation (ssum start-fold + pooled); prsb; rstd."""
                wTsb = st["wTsb"]
                qn_t = st["qn_t"]
                ssum_t = st["ssum_t"]
                pr = psB.tile([128, D], f32, tag="pr")
                # full-width start: clears the bank, writes ssum rows
                nc.tensor.matmul(pr, identE_t, ssum_t, start=True, stop=False, skip_group_check=True)
                for ci, (q0, cnt) in enumerate(QCH):
                    for j in range(BLK):
                        nc.tensor.matmul(
                            pr[32 * j:32 * j + W, :],
                            wTsb[ci][0:cnt, 32 * j:32 * j + W],
                            qn_t[ci][0:cnt, j, :],
                            start=False,
                            stop=(ci == len(QCH) - 1 and j == BLK - 1),
                            tile_position=(0, 32 * j),
                            skip_group_check=True,
                        )
                prsb = mid.tile([128, D], bf16, tag="prsb")
                nc.vector.tensor_copy(prsb, pr)
                sqdump = mid.tile([128, D], bf16, tag="sqdump", bufs=2)
                n2 = mid.tile([128, 1], f32, tag="n2")
                nc.scalar.activation(out=sqdump, in_=pr,
                                     func=mybir.ActivationFunctionType.Square,
                                     bias=0.0, scale=1.0, accum_out=n2)
                lnn = mid.tile([128, 1], f32, tag="lnn")
                nc.scalar.activation(out=lnn, in_=n2,
                                     func=mybir.ActivationFunctionType.Ln,
                                     bias=0.0, scale=1.0)
                rstd = mid.tile([128, 1], f32, tag="rstd")
                nc.scalar.activation(out=rstd, in_=lnn,
                                     func=mybir.ActivationFunctionType.Exp,
                                     bias=0.0, scale=-0.5)
                st["prsb"] = prsb
                st["rstd"] = rstd

            def s3a(b, st):
                """prT via DMA xbar transposes (end-of-iteration)."""
                ptsb = []
                for c in range(DC):
                    t = mid.tile([128, 128], bf16, tag=f"ptsb{c}")
                    nc.sync.dma_start(
                        out=t, in_=st["prsb"][:, 128 * c:128 * (c + 1)],
                        transpose=True)
                    ptsb.append(t)
                st["ptsb"] = ptsb

            def s3b(b, st):
                """logits matmul (transposed layout)."""
                ptsb = st["ptsb"]
                lgT = psA.tile([128, Q], f32, tag="lgT", bufs=2)
                # K=1 zero start: clears the bank
                nc.tensor.matmul(lgT, zcol_t, st["nqh_t"][0:1, :],
                                 start=True, stop=False, skip_group_check=True)
                for c in range(DC):
                    for j in range(BLK):
                        nc.tensor.matmul(
                            lgT[32 * j:32 * j + W, :],
                            ptsb[c][:, 32 * j:32 * j + W],
                            st["qTt"][:, c, j, :],
                            start=False,
                            stop=(c == DC - 1 and j == BLK - 1),
                            tile_position=(0, 32 * j),
                            skip_group_check=True,
                        )
                st["lgT"] = lgT

            def s4(b, st):
                """store transposed logits scaled by rstd (folds proto l2norm)."""
                lgsb = mid.tile([128, Q], bf16, tag="lgsb")
                nc.vector.tensor_scalar_mul(out=lgsb, in0=st["lgT"],
                                            scalar1=st["rstd"])
                nc.gpsimd.dma_start(out=outT[:, b, :], in_=lgsb)

            # software pipeline with dense PE mega-bursts per iteration:
            #   PE order/iter: G_i | M2_{i-1} | M3_{i-2}
            # transposes run as DMA, issued end-of-iteration for overlap
            for _rep in range(reps):
                sts = {}
                for i in range(NBLK + 3):
                    if i < NBLK:
                        sts[i] = {}
                        s1(i, sts[i])
                    if 0 <= i - 1 < NBLK:
                        s2b(i - 1, sts[i - 1])
                    if 0 <= i - 2 < NBLK:
                        s3b(i - 2, sts[i - 2])
                    if 0 <= i - 3 < NBLK:
                        s4(i - 3, sts[i - 3])
                        del sts[i - 3]
                    if i < NBLK:
                        s2a(i, sts[i])
                    if 0 <= i - 1 < NBLK:
                        s3a(i - 1, sts[i - 1])

    nc.finalize()
    return nc


def _get_built(reps=1):
    if reps not in _BUILT:
        _BUILT[reps] = _build(reps)
    return _BUILT[reps]


def _prep_core_inputs(x_shot, x_query, temp):
    """x_shot [EL,W,S,D] f32, x_query [EL,Q,D] f32 -> input map for one core."""
    q2 = QS * x_query                                # [EL, Q, D] f32
    qn = np.ascontiguousarray(q2.transpose(1, 0, 2)).astype(ml_dtypes.float8_e4m3)
    qTr = q2.transpose(2, 0, 1).reshape(DC, 128, NBLK, BLK, Q)
    qTr = np.ascontiguousarray(qTr.transpose(1, 2, 0, 3, 4)).reshape(128, NBLK, DC * BLK * Q)
    qTr = qTr.astype(ml_dtypes.float8_e3m4)

    shot_sum = x_shot.sum(axis=2)                    # [EL, W, D] f32
    mean = shot_sum / S
    # ssum packed: [BLK*W, NBLK, D]; rows 20j+w of block b = shot_sum[4b+j, w]
    ssb = shot_sum.reshape(NBLK, BLK * W, D).transpose(1, 0, 2)
    ssb = np.ascontiguousarray(ssb)
    # mTs[p, b, (c j w)] = -2/QS * mean[4b+j, w, 128c+p]
    m = (-2.0 / QS * mean).reshape(NBLK, BLK, W, DC, 128)
    m = m.transpose(4, 0, 3, 1, 2).reshape(128, NBLK, DC * BLK * W)
    mTs_a = np.ascontiguousarray(m).astype(ml_dtypes.float8_e3m4)

    nq2 = np.einsum("eqd,eqd->eq", x_query.astype(np.float64),
                    x_query.astype(np.float64)).astype(np.float32)   # [EL, Q]
    hi = nq2.astype(ml_dtypes.bfloat16)
    lo = (nq2 - hi.astype(np.float32)).astype(ml_dtypes.bfloat16)
    # nqhl[2j+t, b, q]: hi/lo interleaved per episode j of block b
    nqhl = np.zeros((2 * BLK, NBLK, Q), ml_dtypes.bfloat16)
    for b in range(NBLK):
        for j in range(BLK):
            nqhl[2 * j, b, :] = hi[BLK * b + j]
            nqhl[2 * j + 1, b, :] = lo[BLK * b + j]

    nm2 = np.einsum("ewd,ewd->ew", mean, mean)       # [EL, W] f32
    nm2b = np.ones((128, NBLK), np.float32)          # 1.0 keeps junk rows finite
    for b in range(NBLK):
        for j in range(BLK):
            nm2b[32 * j:32 * j + W, b] = nm2[BLK * b + j]

    # ones8[2j+t, 32j:32j+W] = 1: G start matmul sums hi+lo into rows 32j+w
    ones8 = np.zeros((2 * BLK, 128), ml_dtypes.bfloat16)
    # identE[20j+w, 32j+w] = 1: pr start matmul scatters packed ssum rows
    identE = np.zeros((BLK * W, 128), ml_dtypes.bfloat16)
    for j in range(BLK):
        ones8[2 * j, 32 * j:32 * j + W] = 1
        ones8[2 * j + 1, 32 * j:32 * j + W] = 1
        for w in range(W):
            identE[20 * j + w, 32 * j + w] = 1
    zcol = np.zeros((1, 128), ml_dtypes.bfloat16)

    return {
        "ssum": ssb.astype(ml_dtypes.bfloat16), "qnat": qn, "qT": qTr, "mTs": mTs_a,
        "nq2hl": nqhl, "ones8d": ones8, "identEd": identE, "zcold": zcol,
        "nm2b": nm2b,
    }


def _postprocess(x_query_full, temp, results):
    out = np.empty((E, Q, W), np.float32)
    for i in range(NCORES):
        sl = slice(i * EL, (i + 1) * EL)
        nq2 = np.einsum("eqd,eqd->eq", x_query_full[sl].astype(np.float64),
                        x_query_full[sl].astype(np.float64)).astype(np.float32)
        qscale = (np.float32(temp) / (QS * np.sqrt(nq2)))[:, :, None]  # [EL, Q, 1]
        raw = results[i]["outT"].astype(np.float32).reshape(4, 32, NBLK, Q)
        lg = raw[:, 0:W].transpose(2, 0, 1, 3).reshape(EL, W, Q)  # [e, w, q]
        out[sl] = lg.transpose(0, 2, 1) * qscale
    return out


def _run(x_shot, x_query, temp):
    nc = _get_built()
    in_maps = []
    for i in range(NCORES):
        sl = slice(i * EL, (i + 1) * EL)
        in_maps.append(_prep_core_inputs(x_shot[sl], x_query[sl], temp))
    res = run_bass_kernel_spmd(nc, in_maps, list(range(NCORES)), trace=False)
    return _postprocess(x_query, temp, res.results), res


def kernel(x_shot, x_query, temp):
    x_shot = np.asarray(x_shot, dtype=np.float32)
    x_query = np.asarray(x_query, dtype=np.float32)
    out, _ = _run(x_shot, x_query, np.float32(temp))
    return out
